# revision 2
# baseline (speedup 1.0000x reference)
"""Bidirectional Mamba block on 8 Trainium2 NeuronCores.

Sharding: data-parallel over (direction, batch): core c handles
direction c//4 (0=fwd, 1=bwd) and batch c%4.  The final projection is
linear over the concat([out_fwd, out_bwd]) axis, so each core applies its
direction's half of proj_W and the host sums the two partial outputs
(plus proj_b).  Zero cross-core communication.

Per-core layout is "d-major": tiles are [128 partitions = channel slice,
free = sequence].  The selective-scan recurrence h_t = dA_t*h_{t-1} + dBu_t
runs on the Vector engine's tensor_tensor_scan (prefix scan along the free
dim), once per (state s, channel tile): dA_s = exp(A[:, s] * delta).
"""
import sys

sys.path.insert(0, "/opt/trn_rl_repo")

import numpy as np

import concourse.bass as bass
import concourse.tile as tile
from concourse import mybir
from concourse.bass_utils import run_bass_kernel_spmd
from concourse.vector_clock import ScopedClock

# ---------------------------------------------------------------- shapes
D_MODEL = 768
D_STATE = 16
D_CONV = 4
D_INNER = 1536
DT_RANK = 48
B, L = 4, 1024

P = 128
NDT = D_INNER // P      # 12  channel tiles
NK = D_MODEL // P       # 6   d_model contraction tiles
NE = 2 * D_INNER // P   # 24  in_proj output tiles
NDM = D_MODEL // P      # 6   d_model output tiles
TH = 2                  # two 512-wide t-halves for matmuls
F32 = mybir.dt.float32
AF = mybir.ActivationFunctionType
OP = mybir.AluOpType

N_CORES = 8

BF16 = mybir.dt.bfloat16
F32R = mybir.dt.float32r
SCAN_BF16 = True   # bf16 operands for the scan stage (2x DVE modes)
Y_BF16 = True      # accumulate y in bf16 too (cheaper adds, more error)
MM_DT = "f32r"     # matmul operand dtype: f32 (4 cyc/row), f32r/bf16 (1)
WDT = {"f32": F32, "f32r": F32R, "bf16": BF16}[MM_DT]


def _f32(ap):
    """View a WDT-typed AP as plain fp32 for vector/scalar-engine reads."""
    return ap.bitcast(F32) if MM_DT == "f32r" else ap


MAX_WAITS_PER_INST = 1


class SplitDrainTileContext(tile.TileContext):
    """Walrus in this container rejects >1 sem-wait per instruction; the stock
    kernel-tail drain carries one wait per active processor.  Split them into
    a chain of single-wait SP NOPs."""

    def _drain_and_barrier(self, tick_clock, wait_clock):
        nc = self.nc
        carrier = nc.sync.nop(nofuse=True)
        wait_clock.add_sem_waits(
            carrier.ins, ScopedClock({None: tick_clock.global_clock})
        )
        si = carrier.ins.sync_info
        waits = list(si.on_wait) if si is not None and si.on_wait else []
        if len(waits) > MAX_WAITS_PER_INST:
            carrier.ins.sync_info = mybir.SyncInfo(
                on_wait=waits[:MAX_WAITS_PER_INST], on_update=[]
            )
            rest = waits[MAX_WAITS_PER_INST:]
            for i in range(0, len(rest), MAX_WAITS_PER_INST):
                extra = nc.sync.nop(nofuse=True)
                extra.ins.sync_info = mybir.SyncInfo(
                    on_wait=rest[i : i + MAX_WAITS_PER_INST], on_update=[]
                )
        nc.sync.drain()
        nc.all_engine_barrier()
        assert self.sems is not None
        popped = nc._tile_sem_poison_stack.pop()
        assert popped is self._sem_poison
        nc.clear_and_free_semaphores(list(self.sems.allocated().values()))


def _split_multi_waits(nc):
    """Walrus here accepts at most one sem-wait per instruction.  Tile's
    wait-assignment can attach several (e.g. a matmul waiting on weight DMA +
    rhs producer + PSUM release).  Hoist all but the last wait onto same-
    engine NOPs inserted immediately before the instruction."""
    n = 0
    for fn in nc.m.functions:
        for bb in fn.blocks:
            out = []
            for ins in bb.instructions:
                si = ins.sync_info
                waits = list(si.on_wait) if si is not None and si.on_wait else []
                if len(waits) > 1:
                    for wv in waits[:-1]:
                        nop = mybir.InstNoOp(name=f"wsplit_{n}", ins=[], outs=[])
                        n += 1
                        nop.engine = ins.engine
                        nop.sync_info = mybir.SyncInfo(on_wait=[wv], on_update=[])
                        out.append(nop)
                    ins.sync_info = mybir.SyncInfo(
                        on_wait=[waits[-1]], on_update=list(si.on_update or [])
                    )
                out.append(ins)
            bb.instructions = out
    return n


def _col_block_ap(handle, width, col0, ncols, kcount):
    """AP reading rows [0:128*kcount) x cols [col0:col0+ncols) of a [R, width]
    DRAM tensor as a [128, kcount*ncols] tile (k-blocks side by side)."""
    base = handle[:]
    return bass.AP(
        tensor=base.tensor,
        offset=col0,
        ap=[[width, P], [P * width, kcount], [1, ncols]],
    )


def _bcast_ap(src):
    """AP that reads a [L]-row and broadcasts it across 128 partitions."""
    return bass.AP(
        tensor=src.tensor, offset=src.offset, ap=[[0, P]] + list(src.ap)
    )


# ---------------------------------------------------------------- program
def _build_program(split_waits=True):
    nc = bass.Bass()

    di = lambda name, shape: nc.dram_tensor(name, shape, F32, kind="ExternalInput")
    dw = lambda name, shape: nc.dram_tensor(name, shape, WDT, kind="ExternalInput")
    xT = dw("xT", [D_MODEL, L])
    inWT = dw("inWT", [D_MODEL, 2 * D_INNER])
    convw = di("convw", [P, NDT * D_CONV])   # host pre-tiled [(p), (dt k)]
    convb = di("convb", [P, NDT])
    xprojWT = dw("xprojWT", [D_INNER, DT_RANK + 2 * D_STATE])
    dtWT = dw("dtWT", [DT_RANK, D_INNER])
    dtb = di("dtb", [P, NDT])
    Aarr = di("Aarr", [P, NDT * D_STATE])    # host pre-tiled -exp(A_log)
    Dp = di("Dp", [P, NDT])
    outWT = dw("outWT", [D_INNER, D_MODEL])
    projHT = dw("projHT", [D_MODEL, D_MODEL])

    part = nc.dram_tensor("part", [D_MODEL, L], F32, kind="ExternalOutput")

    xc_park = nc.dram_tensor("xc_park", [D_INNER, L], F32)
    sz_park = nc.dram_tensor("sz_park", [D_INNER, L], F32)
    bc_park = nc.dram_tensor("bc_park", [2 * D_STATE, L],
                             BF16 if SCAN_BF16 else F32)

    with SplitDrainTileContext(nc) as tc:
        from contextlib import ExitStack

        with ExitStack() as g:
            consts = g.enter_context(tc.tile_pool(name="consts", bufs=1))
            psum = g.enter_context(tc.tile_pool(name="psum", bufs=1, space="PSUM"))

            # -------- constants
            convw_sb = consts.tile([P, NDT * D_CONV], F32, name="convw")
            nc.sync.dma_start(out=convw_sb[:], in_=convw[:])
            convb_sb = consts.tile([P, NDT], F32, name="convb")
            nc.sync.dma_start(out=convb_sb[:], in_=convb[:])
            dtb_sb = consts.tile([P, NDT], F32, name="dtb")
            nc.sync.dma_start(out=dtb_sb[:], in_=dtb[:])
            A_sb = consts.tile([P, NDT * D_STATE], F32, name="A")
            nc.sync.dma_start(out=A_sb[:], in_=Aarr[:])
            D_sb = consts.tile([P, NDT], F32, name="D")
            nc.sync.dma_start(out=D_sb[:], in_=Dp[:])

            e1 = ExitStack()  # [start .. du-end]
            e0 = ExitStack()  # [start .. conv-end]
            e2 = ExitStack()  # [dt_proj .. scan-end]
            e3 = ExitStack()  # [scan .. scan-end]
            e4 = ExitStack()  # [scan .. gate-end]
            e5 = ExitStack()  # [gate .. end]

            wpool = e1.enter_context(tc.tile_pool(name="w", bufs=6))
            xcs_pool = e1.enter_context(tc.tile_pool(name="xcs", bufs=4))
            xdbl_pool = e1.enter_context(tc.tile_pool(name="xdbl", bufs=1))
            xt_pool = e0.enter_context(tc.tile_pool(name="xt", bufs=1))
            xi_pool = e0.enter_context(tc.tile_pool(name="xi", bufs=1))
            sz_tmp = e0.enter_context(tc.tile_pool(name="sztmp", bufs=3))
            cacc_pool = e0.enter_context(tc.tile_pool(name="cacc", bufs=2))

            # ---- load xT
            xt_sb = []
            for k in range(NK):
                t = xt_pool.tile([P, L], WDT, name=f"xt{k}", tag=f"xt{k}")
                nc.sync.dma_start(out=t[:], in_=xT[k * P : (k + 1) * P, :])
                xt_sb.append(t)

            # ---- in_proj: xzT[e,t] = sum_k inWT[k,e]^T x[k,t]
            xi_sb = [
                xi_pool.tile([P, L + 3], F32, name=f"xi{d}", tag=f"xi{d}")
                for d in range(NDT)
            ]
            for d in range(NDT):
                nc.vector.memset(xi_sb[d][:, 0:3], 0.0)

            for e in range(NE):
                we = wpool.tile([P, NK * P], WDT, name="we", tag="we", bufs=3)
                nc.sync.dma_start(
                    out=we[:],
                    in_=_col_block_ap(inWT, 2 * D_INNER, e * P, P, NK),
                )
                for th in range(TH):
                    ps = psum.tile([P, 512], F32, name="mm", tag="mm", bufs=3)
                    for k in range(NK):
                        nc.tensor.matmul(
                            ps[:],
                            we[:, k * P : (k + 1) * P],
                            xt_sb[k][:, th * 512 : (th + 1) * 512],
                            start=(k == 0),
                            stop=(k == NK - 1),
                        )
                    if e < NDT:
                        nc.scalar.copy(
                            xi_sb[e][:, 3 + th * 512 : 3 + (th + 1) * 512], ps[:]
                        )
                    else:
                        d = e - NDT
                        sz = sz_tmp.tile([P, 512], F32, name="sz", tag="sz")
                        nc.scalar.activation(sz[:], ps[:], AF.Silu)
                        nc.sync.dma_start(
                            out=sz_park[
                                d * P : (d + 1) * P, th * 512 : (th + 1) * 512
                            ],
                            in_=sz[:],
                        )

            # ---- conv + silu -> xc (streamed to DRAM), x_proj accumulates
            # into two PSUM banks held across the d loop
            xdbl_sb = xdbl_pool.tile([P, L], WDT, name="xdbl")
            NR = DT_RANK + 2 * D_STATE  # 80
            psx = [
                psum.tile([P, 512], F32, name=f"mmx{th}", tag=f"mmx{th}")
                for th in range(TH)
            ]
            xc_sb = []
            for d in range(NDT):
                acc = cacc_pool.tile([P, L], F32, name="cacc", tag="cacc")
                nc.vector.tensor_scalar(
                    acc[:],
                    xi_sb[d][:, 0:L],
                    convw_sb[:, 4 * d : 4 * d + 1],
                    convb_sb[:, d : d + 1],
                    op0=OP.mult,
                    op1=OP.add,
                )
                for k in range(1, D_CONV):
                    nc.vector.scalar_tensor_tensor(
                        acc[:],
                        xi_sb[d][:, k : k + L],
                        convw_sb[:, 4 * d + k : 4 * d + k + 1],
                        acc[:],
                        op0=OP.mult,
                        op1=OP.add,
                    )
                xc_t = xcs_pool.tile([P, L], WDT, name=f"xct{d}", tag=f"xct{d}", bufs=1)
                xc_sb.append(xc_t)
                nc.scalar.activation(xc_t[:], acc[:], AF.Silu)

                wx = wpool.tile([P, NR], WDT, name="wx", tag="wx")
                nc.sync.dma_start(out=wx[:], in_=xprojWT[d * P : (d + 1) * P, :])
                for th in range(TH):
                    nc.tensor.matmul(
                        psx[th][:NR, :],
                        wx[:],
                        xc_t[:, th * 512 : (th + 1) * 512],
                        start=(d == 0),
                        stop=(d == NDT - 1),
                    )
                nc.sync.dma_start(
                    out=xc_park[d * P : (d + 1) * P, :], in_=_f32(xc_t[:])
                )
            for th in range(TH):
                nc.scalar.copy(
                    xdbl_sb[:NR, th * 512 : (th + 1) * 512], psx[th][:NR, :]
                )

            # park raw B/C rows for broadcast reload
            if SCAN_BF16:
                # ACT partition-window rule: 32-aligned base, <=32 rows per
                # access here.  Two copies cover B (rows 48..63) and C
                # (rows 64..79).
                bcrows = xdbl_pool.tile([64, L], BF16, name="bcrows", bufs=1)
                nc.scalar.copy(bcrows[0:32, :], _f32(xdbl_sb[32:64, :]))
                nc.scalar.copy(bcrows[32:48, :], _f32(xdbl_sb[64:80, :]))
                nc.sync.dma_start(out=bc_park[:], in_=bcrows[16:48, :])
            else:
                nc.sync.dma_start(
                    out=bc_park[:],
                    in_=xdbl_sb[DT_RANK : DT_RANK + 2 * D_STATE, :],
                )

            e0.close()  # free xt/xi/sztmp/cacc

            delta_pool = e2.enter_context(tc.tile_pool(name="delta", bufs=1, side="right"))
            du_pool = e2.enter_context(tc.tile_pool(name="du", bufs=1, side="right"))
            delta_sb = [
                delta_pool.tile([P, L], F32, name=f"dl{d}", tag=f"dl{d}")
                for d in range(NDT)
            ]
            SDT = BF16 if SCAN_BF16 else F32
            du_sb = [
                du_pool.tile([P, L], SDT, name=f"du{d}", tag=f"du{d}")
                for d in range(NDT)
            ]

            # ---- dt_proj + softplus -> delta
            # (no softplus in this build's ACT tables: use ln(1 + exp(z+b)),
            # both funcs live in the natural_log_exp_and_others set)
            sptmp_pool = e1.enter_context(tc.tile_pool(name="sptmp", bufs=2))
            wdt = wpool.tile([P, D_INNER], WDT, name="wdt", tag="wdt", bufs=1)
            nc.sync.dma_start(out=wdt[:DT_RANK, :], in_=dtWT[:])
            for d in range(NDT):
                for th in range(TH):
                    ps = psum.tile([P, 512], F32, name="mm", tag="mm", bufs=3)
                    nc.tensor.matmul(
                        ps[:],
                        wdt[:DT_RANK, d * P : (d + 1) * P],
                        xdbl_sb[:DT_RANK, th * 512 : (th + 1) * 512],
                    )
                    u = sptmp_pool.tile([P, 512], F32, name="spu", tag="spu")
                    nc.scalar.activation(
                        u[:], ps[:], AF.Exp, bias=dtb_sb[:, d : d + 1]
                    )
                    nc.scalar.activation(
                        delta_sb[d][:, th * 512 : (th + 1) * 512],
                        u[:],
                        AF.Ln,
                        bias=1.0,
                    )

            # ---- du = delta * xc (xc still resident)
            for d in range(NDT):
                nc.vector.tensor_mul(du_sb[d][:], delta_sb[d][:], _f32(xc_sb[d][:]))

            e1.close()  # free w/xcs/xdbl

            # -------- selective scan, s-outer / d-inner
            ypool = e4.enter_context(tc.tile_pool(name="y", bufs=1))
            bcpool = e3.enter_context(tc.tile_pool(name="bc", bufs=2))
            stp = e3.enter_context(tc.tile_pool(name="scantmp", bufs=2))
            hcp = e3.enter_context(tc.tile_pool(name="hCp", bufs=1))

            YDT = BF16 if Y_BF16 else F32
            y_sb = [
                ypool.tile([P, L], YDT, name=f"y{d}", tag=f"y{d}")
                for d in range(NDT)
            ]
            for s in range(D_STATE):
                B_bc = bcpool.tile([P, L], SDT, name="Bbc", tag="Bbc")
                nc.gpsimd.dma_start(out=B_bc[:], in_=_bcast_ap(bc_park[s]))
                C_bc = bcpool.tile([P, L], SDT, name="Cbc", tag="Cbc")
                nc.gpsimd.dma_start(
                    out=C_bc[:], in_=_bcast_ap(bc_park[D_STATE + s])
                )
                for d in range(NDT):
                    dA = stp.tile([P, L], SDT, name="dA", tag="dA")
                    col = d * D_STATE + s
                    nc.scalar.activation(
                        dA[:], delta_sb[d][:], AF.Exp,
                        scale=A_sb[:, col : col + 1],
                    )
                    b = stp.tile([P, L], SDT, name="b", tag="b")
                    # gpsimd runs TTs ~3x slower than DVE (0.42 impl
                    # efficiency, no 2x bf16 mode) but DVE is the critical
                    # path: offload b always and hC for s<=9 to balance
                    # (vec ~= gps ~= 610us predicted).
                    nc.gpsimd.tensor_mul(b[:], du_sb[d][:], B_bc[:])
                    h = stp.tile([P, L], SDT, name="h", tag="h")
                    nc.vector.tensor_tensor_scan(
                        h[:], dA[:], b[:], 0.0, op0=OP.mult, op1=OP.add
                    )
                    if s == 0:
                        nc.vector.tensor_mul(y_sb[d][:], h[:], C_bc[:])
                    else:
                        hC = hcp.tile([P, L], SDT, name="hC", tag="hC")
                        eng = nc.gpsimd if s <= 9 else nc.vector
                        eng.tensor_mul(hC[:], h[:], C_bc[:])
                        nc.vector.tensor_add(y_sb[d][:], y_sb[d][:], hC[:])

            e3.close()  # free bc/scantmp/hC
            e2.close()  # free delta/du

            # -------- skip + gate: y3 = (xc*D + y) * silu(z), then projections
            stream = e5.enter_context(tc.tile_pool(name="stream", bufs=2))
            y3pool = e5.enter_context(tc.tile_pool(name="y3", bufs=1))
            mopool = e5.enter_context(tc.tile_pool(name="mo", bufs=1))
            w2pool = e5.enter_context(tc.tile_pool(name="w2", bufs=6))
            otmp = e5.enter_context(tc.tile_pool(name="otmp", bufs=3))

            y3_sb = [
                y3pool.tile([P, L], WDT, name=f"y3{d}", tag=f"y3{d}")
                for d in range(NDT)
            ]
            for d in range(NDT):
                xc_t = stream.tile([P, L], F32, name="xcs", tag="xcs")
                nc.sync.dma_start(
                    out=xc_t[:], in_=xc_park[d * P : (d + 1) * P, :]
                )
                sz_t = stream.tile([P, L], F32, name="szs", tag="szs")
                nc.sync.dma_start(
                    out=sz_t[:], in_=sz_park[d * P : (d + 1) * P, :]
                )
                y2 = stream.tile([P, L], F32, name="y2", tag="y2")
                nc.vector.scalar_tensor_tensor(
                    y2[:], xc_t[:], D_sb[:, d : d + 1], y_sb[d][:],
                    op0=OP.mult, op1=OP.add,
                )
                nc.vector.tensor_mul(y3_sb[d][:], y2[:], sz_t[:])

            # ---- out_proj: mo[m,t] = sum_d outWT[d,m]^T y3[d,t]
            mo_sb = [
                mopool.tile([P, L], WDT, name=f"mo{m}", tag=f"mo{m}")
                for m in range(NDM)
            ]
            for m in range(NDM):
                wo = w2pool.tile([P, NDT * P], WDT, name="wo", tag="wo", bufs=2)
                nc.sync.dma_start(
                    out=wo[:], in_=_col_block_ap(outWT, D_MODEL, m * P, P, NDT)
                )
                for th in range(TH):
                    ps = psum.tile([P, 512], F32, name="mm", tag="mm", bufs=3)
                    for d in range(NDT):
                        nc.tensor.matmul(
                            ps[:],
                            wo[:, d * P : (d + 1) * P],
                            y3_sb[d][:, th * 512 : (th + 1) * 512],
                            start=(d == 0),
                            stop=(d == NDT - 1),
                        )
                    nc.scalar.copy(mo_sb[m][:, th * 512 : (th + 1) * 512], ps[:])

            # ---- final half-projection
            for m2 in range(NDM):
                wp = w2pool.tile([P, NDM * P], WDT, name="wp", tag="wp", bufs=2)
                nc.sync.dma_start(
                    out=wp[:], in_=_col_block_ap(projHT, D_MODEL, m2 * P, P, NDM)
                )
                for th in range(TH):
                    ps = psum.tile([P, 512], F32, name="mm", tag="mm", bufs=3)
                    for m in range(NDM):
                        nc.tensor.matmul(
                            ps[:],
                            wp[:, m * P : (m + 1) * P],
                            mo_sb[m][:, th * 512 : (th + 1) * 512],
                            start=(m == 0),
                            stop=(m == NDM - 1),
                        )
                    o = otmp.tile([P, 512], F32, name="o", tag="o")
                    nc.scalar.copy(o[:], ps[:])
                    nc.sync.dma_start(
                        out=part[m2 * P : (m2 + 1) * P, th * 512 : (th + 1) * 512],
                        in_=o[:],
                    )

            e5.close()
            e4.close()

    if split_waits:
        _split_multi_waits(nc)
    return nc


_NC_CACHE = None


def _get_program():
    global _NC_CACHE
    if _NC_CACHE is None:
        _NC_CACHE = _build_program()
    return _NC_CACHE


# ---------------------------------------------------------------- host glue
def _core_inputs(x_b, pfx, inputs):
    """Build the per-core in_map for one (direction, batch)."""
    c = np.ascontiguousarray
    inW = inputs[pfx + "_inW"]
    convw = inputs[pfx + "_convw"]
    convb = inputs[pfx + "_convb"]
    xprojW = inputs[pfx + "_xprojW"]
    dtW = inputs[pfx + "_dtW"]
    dtb = inputs[pfx + "_dtb"]
    Alog = inputs[pfx + "_Alog"]
    Dv = inputs[pfx + "_D"]
    outW = inputs[pfx + "_outW"]

    A = -np.exp(np.asarray(Alog, np.float32))  # (D_INNER, D_STATE)

    def ptile(v, inner):  # (D_INNER, inner) -> (P, NDT*inner)
        v = np.asarray(v, np.float32).reshape(NDT, P, inner)
        return c(v.transpose(1, 0, 2).reshape(P, NDT * inner))

    return {
        "xT": c(np.asarray(x_b, np.float32).T),
        "inWT": c(np.asarray(inW, np.float32).T),
        "convw": ptile(convw, D_CONV),
        "convb": ptile(np.asarray(convb).reshape(-1, 1), 1),
        "xprojWT": c(np.asarray(xprojW, np.float32).T),
        "dtWT": c(np.asarray(dtW, np.float32).T),
        "dtb": ptile(np.asarray(dtb).reshape(-1, 1), 1),
        "Aarr": ptile(A, D_STATE),
        "Dp": ptile(np.asarray(Dv).reshape(-1, 1), 1),
        "outWT": c(np.asarray(outW, np.float32).T),
    }


def _build_in_maps(inputs):
    x = np.asarray(inputs["x"], np.float32)
    projW = np.asarray(inputs["proj_W"], np.float32)
    in_maps = []
    for core in range(N_CORES):
        direction, b = core // B, core % B
        if direction == 0:
            m = _core_inputs(x[b], "fwd", inputs)
            m["projHT"] = np.ascontiguousarray(projW[:, :D_MODEL].T.astype(np.float32))
        else:
            m = _core_inputs(x[b, ::-1], "bwd", inputs)
            m["projHT"] = np.ascontiguousarray(projW[:, D_MODEL:].T.astype(np.float32))
        in_maps.append(m)
    return in_maps


def kernel(**inputs):
    projb = np.asarray(inputs["proj_b"], np.float32)
    in_maps = _build_in_maps(inputs)
    nc = _get_program()
    res = run_bass_kernel_spmd(nc, in_maps, list(range(N_CORES)))

    out = np.empty((B, L, D_MODEL), np.float32)
    for b in range(B):
        pf = res.results[b]["part"]          # (D_MODEL, L)
        pb = res.results[B + b]["part"]      # (D_MODEL, L) in flipped time
        out[b] = (pf + pb[:, ::-1]).T + projb[None, :]
    return out



# revision 15
# speedup vs baseline: 1.4472x; 1.4472x over previous
"""Bidirectional Mamba block on 8 Trainium2 NeuronCores.

Sharding: data-parallel over (direction, batch): core c handles
direction c//4 (0=fwd, 1=bwd) and batch c%4.  The final projection is
linear over the concat([out_fwd, out_bwd]) axis, so each core applies its
direction's half of proj_W and the host sums the two partial outputs
(plus proj_b).  Zero cross-core communication.

Per-core layout is "d-major": tiles are [128 partitions = channel slice,
free = sequence].  The selective-scan recurrence h_t = dA_t*h_{t-1} + dBu_t
runs on the Vector engine's tensor_tensor_scan (prefix scan along the free
dim), once per (state s, channel tile): dA_s = exp(A[:, s] * delta).
"""
import sys

sys.path.insert(0, "/opt/trn_rl_repo")

import numpy as np

import concourse.bass as bass
import concourse.tile as tile
from concourse import mybir
from concourse.bass_utils import run_bass_kernel_spmd
from concourse.vector_clock import ScopedClock

# ---------------------------------------------------------------- shapes
D_MODEL = 768
D_STATE = 16
D_CONV = 4
D_INNER = 1536
DT_RANK = 48
B, L = 4, 1024

P = 128
NDT = D_INNER // P      # 12  channel tiles
NK = D_MODEL // P       # 6   d_model contraction tiles
NE = 2 * D_INNER // P   # 24  in_proj output tiles
NDM = D_MODEL // P      # 6   d_model output tiles
TH = 2                  # two 512-wide t-halves for matmuls
F32 = mybir.dt.float32
AF = mybir.ActivationFunctionType
OP = mybir.AluOpType

N_CORES = 8

BF16 = mybir.dt.bfloat16
F32R = mybir.dt.float32r
SCAN_BF16 = True   # bf16 operands for the scan stage (2x DVE modes)
Y_BF16 = True      # accumulate y in bf16 too (cheaper adds, more error)
MM_DT = "f32r"     # matmul operand dtype: f32 (4 cyc/row), f32r/bf16 (1)
WDT = {"f32": F32, "f32r": F32R, "bf16": BF16}[MM_DT]


def _f32(ap):
    """View a WDT-typed AP as plain fp32 for vector/scalar-engine reads."""
    return ap.bitcast(F32) if MM_DT == "f32r" else ap


MAX_WAITS_PER_INST = 1


class SplitDrainTileContext(tile.TileContext):
    """Walrus in this container rejects >1 sem-wait per instruction; the stock
    kernel-tail drain carries one wait per active processor.  Split them into
    a chain of single-wait SP NOPs."""

    def _drain_and_barrier(self, tick_clock, wait_clock):
        nc = self.nc
        carrier = nc.sync.nop(nofuse=True)
        wait_clock.add_sem_waits(
            carrier.ins, ScopedClock({None: tick_clock.global_clock})
        )
        si = carrier.ins.sync_info
        waits = list(si.on_wait) if si is not None and si.on_wait else []
        if len(waits) > MAX_WAITS_PER_INST:
            carrier.ins.sync_info = mybir.SyncInfo(
                on_wait=waits[:MAX_WAITS_PER_INST], on_update=[]
            )
            rest = waits[MAX_WAITS_PER_INST:]
            for i in range(0, len(rest), MAX_WAITS_PER_INST):
                extra = nc.sync.nop(nofuse=True)
                extra.ins.sync_info = mybir.SyncInfo(
                    on_wait=rest[i : i + MAX_WAITS_PER_INST], on_update=[]
                )
        nc.sync.drain()
        nc.all_engine_barrier()
        assert self.sems is not None
        popped = nc._tile_sem_poison_stack.pop()
        assert popped is self._sem_poison
        nc.clear_and_free_semaphores(list(self.sems.allocated().values()))


def _split_multi_waits(nc):
    """Walrus here accepts at most one sem-wait per instruction.  Tile's
    wait-assignment can attach several (e.g. a matmul waiting on weight DMA +
    rhs producer + PSUM release).  Hoist all but the last wait onto same-
    engine NOPs inserted immediately before the instruction."""
    n = 0
    for fn in nc.m.functions:
        for bb in fn.blocks:
            out = []
            for ins in bb.instructions:
                si = ins.sync_info
                waits = list(si.on_wait) if si is not None and si.on_wait else []
                if len(waits) > 1:
                    for wv in waits[:-1]:
                        nop = mybir.InstNoOp(name=f"wsplit_{n}", ins=[], outs=[])
                        n += 1
                        nop.engine = ins.engine
                        nop.sync_info = mybir.SyncInfo(on_wait=[wv], on_update=[])
                        out.append(nop)
                    ins.sync_info = mybir.SyncInfo(
                        on_wait=[waits[-1]], on_update=list(si.on_update or [])
                    )
                out.append(ins)
            bb.instructions = out
    return n


def _col_block_ap(handle, width, col0, ncols, kcount):
    """AP reading rows [0:128*kcount) x cols [col0:col0+ncols) of a [R, width]
    DRAM tensor as a [128, kcount*ncols] tile (k-blocks side by side)."""
    base = handle[:]
    return bass.AP(
        tensor=base.tensor,
        offset=col0,
        ap=[[width, P], [P * width, kcount], [1, ncols]],
    )


def _bcast_ap(src):
    """AP that reads a [L]-row and broadcasts it across 128 partitions."""
    return bass.AP(
        tensor=src.tensor, offset=src.offset, ap=[[0, P]] + list(src.ap)
    )


# ---------------------------------------------------------------- program
def _build_program(split_waits=True):
    nc = bass.Bass()

    di = lambda name, shape: nc.dram_tensor(name, shape, F32, kind="ExternalInput")
    dw = lambda name, shape: nc.dram_tensor(name, shape, WDT, kind="ExternalInput")
    xT = dw("xT", [D_MODEL, L])
    inWT = dw("inWT", [D_MODEL, 2 * D_INNER])
    convw = di("convw", [P, NDT * D_CONV])   # host pre-tiled [(p), (dt k)]
    convb = di("convb", [P, NDT])
    # bf16: must match the (bf16) xc moving operand — walrus rejects
    # f32r-stationary x bf16-moving matmuls.
    xprojWT = nc.dram_tensor(
        "xprojWT", [D_INNER, DT_RANK + 2 * D_STATE], BF16, kind="ExternalInput"
    )
    dtWT = dw("dtWT", [DT_RANK, D_INNER])
    dtb = di("dtb", [P, NDT])
    Aarr = di("Aarr", [P, NDT * D_STATE])    # host pre-tiled -exp(A_log)
    Dp = di("Dp", [P, NDT])
    outWT = dw("outWT", [D_INNER, D_MODEL])
    projHT = dw("projHT", [D_MODEL, D_MODEL])

    part = nc.dram_tensor("part", [D_MODEL, L], F32, kind="ExternalOutput")

    bc_park = nc.dram_tensor("bc_park", [2 * D_STATE, L],
                             BF16 if SCAN_BF16 else F32)

    with SplitDrainTileContext(nc) as tc:
        from contextlib import ExitStack

        with ExitStack() as g:
            consts = g.enter_context(tc.tile_pool(name="consts", bufs=1))
            psum = g.enter_context(tc.tile_pool(name="psum", bufs=1, space="PSUM"))

            # -------- constants
            convw_sb = consts.tile([P, NDT * D_CONV], F32, name="convw")
            nc.sync.dma_start(out=convw_sb[:], in_=convw[:])
            convb_sb = consts.tile([P, NDT], F32, name="convb")
            nc.sync.dma_start(out=convb_sb[:], in_=convb[:])
            dtb_sb = consts.tile([P, NDT], F32, name="dtb")
            nc.sync.dma_start(out=dtb_sb[:], in_=dtb[:])
            A_sb = consts.tile([P, NDT * D_STATE], F32, name="A")
            nc.sync.dma_start(out=A_sb[:], in_=Aarr[:])
            D_sb = consts.tile([P, NDT], F32, name="D")
            nc.sync.dma_start(out=D_sb[:], in_=Dp[:])

            e1 = ExitStack()  # [start .. du-end]
            e0 = ExitStack()  # [start .. conv-end]
            e2 = ExitStack()  # [dt_proj .. scan-end]
            e3 = ExitStack()  # [scan .. scan-end]
            e4 = ExitStack()  # [scan .. gate-end]
            e5 = ExitStack()  # [gate .. end]

            # xc and silu(z) stay resident in SBUF as bf16 until the gate
            # stage (e4 scope) instead of round-tripping through DRAM.
            # Allocated before the e0/e1 pools: releases must be LIFO.
            xcs_pool = e4.enter_context(tc.tile_pool(name="xcs", bufs=1))
            sz_pool = e4.enter_context(tc.tile_pool(name="sz", bufs=1))
            wpool = e1.enter_context(tc.tile_pool(name="w", bufs=6))
            xdbl_pool = e1.enter_context(tc.tile_pool(name="xdbl", bufs=1))
            xt_pool = e0.enter_context(tc.tile_pool(name="xt", bufs=1))
            xi_pool = e0.enter_context(tc.tile_pool(name="xi", bufs=1))
            cacc_pool = e0.enter_context(tc.tile_pool(name="cacc", bufs=2))

            # ---- load xT
            xt_sb = []
            for k in range(NK):
                t = xt_pool.tile([P, L], WDT, name=f"xt{k}", tag=f"xt{k}")
                nc.sync.dma_start(out=t[:], in_=xT[k * P : (k + 1) * P, :])
                xt_sb.append(t)

            # ---- in_proj: xzT[e,t] = sum_k inWT[k,e]^T x[k,t]
            xi_sb = [
                xi_pool.tile([P, L + 3], F32, name=f"xi{d}", tag=f"xi{d}")
                for d in range(NDT)
            ]
            for d in range(NDT):
                nc.vector.memset(xi_sb[d][:, 0:3], 0.0)
            sz_sb = [
                sz_pool.tile([P, L], BF16, name=f"szr{d}", tag=f"szr{d}")
                for d in range(NDT)
            ]

            for e in range(NE):
                we = wpool.tile([P, NK * P], WDT, name="we", tag="we", bufs=3)
                nc.sync.dma_start(
                    out=we[:],
                    in_=_col_block_ap(inWT, 2 * D_INNER, e * P, P, NK),
                )
                for th in range(TH):
                    ps = psum.tile([P, 512], F32, name="mm", tag="mm", bufs=3)
                    for k in range(NK):
                        nc.tensor.matmul(
                            ps[:],
                            we[:, k * P : (k + 1) * P],
                            xt_sb[k][:, th * 512 : (th + 1) * 512],
                            start=(k == 0),
                            stop=(k == NK - 1),
                        )
                    if e < NDT:
                        nc.scalar.copy(
                            xi_sb[e][:, 3 + th * 512 : 3 + (th + 1) * 512], ps[:]
                        )
                    else:
                        d = e - NDT
                        nc.scalar.activation(
                            sz_sb[d][:, th * 512 : (th + 1) * 512], ps[:],
                            AF.Silu,
                        )

            # ---- conv + silu -> xc (streamed to DRAM), x_proj accumulates
            # into two PSUM banks held across the d loop
            xdbl_sb = xdbl_pool.tile([P, L], WDT, name="xdbl")
            NR = DT_RANK + 2 * D_STATE  # 80
            psx = [
                psum.tile([P, 512], F32, name=f"mmx{th}", tag=f"mmx{th}")
                for th in range(TH)
            ]
            xc_sb = []
            for d in range(NDT):
                acc = cacc_pool.tile([P, L], F32, name="cacc", tag="cacc")
                nc.vector.tensor_scalar(
                    acc[:],
                    xi_sb[d][:, 0:L],
                    convw_sb[:, 4 * d : 4 * d + 1],
                    convb_sb[:, d : d + 1],
                    op0=OP.mult,
                    op1=OP.add,
                )
                for k in range(1, D_CONV):
                    nc.vector.scalar_tensor_tensor(
                        acc[:],
                        xi_sb[d][:, k : k + L],
                        convw_sb[:, 4 * d + k : 4 * d + k + 1],
                        acc[:],
                        op0=OP.mult,
                        op1=OP.add,
                    )
                xc_t = xcs_pool.tile([P, L], BF16, name=f"xct{d}", tag=f"xct{d}")
                xc_sb.append(xc_t)
                nc.scalar.activation(xc_t[:], acc[:], AF.Silu)

                wx = wpool.tile([P, NR], BF16, name="wx", tag="wx")
                nc.sync.dma_start(out=wx[:], in_=xprojWT[d * P : (d + 1) * P, :])
                for th in range(TH):
                    nc.tensor.matmul(
                        psx[th][:NR, :],
                        wx[:],
                        xc_t[:, th * 512 : (th + 1) * 512],
                        start=(d == 0),
                        stop=(d == NDT - 1),
                    )
            for th in range(TH):
                nc.scalar.copy(
                    xdbl_sb[:NR, th * 512 : (th + 1) * 512], psx[th][:NR, :]
                )

            # park raw B/C rows for broadcast reload
            if SCAN_BF16:
                # ACT partition-window rule: 32-aligned base, <=32 rows per
                # access here.  Two copies cover B (rows 48..63) and C
                # (rows 64..79).
                bcrows = xdbl_pool.tile([64, L], BF16, name="bcrows", bufs=1)
                nc.scalar.copy(bcrows[0:32, :], _f32(xdbl_sb[32:64, :]))
                nc.scalar.copy(bcrows[32:48, :], _f32(xdbl_sb[64:80, :]))
                nc.sync.dma_start(out=bc_park[:], in_=bcrows[16:48, :])
            else:
                nc.sync.dma_start(
                    out=bc_park[:],
                    in_=xdbl_sb[DT_RANK : DT_RANK + 2 * D_STATE, :],
                )

            e0.close()  # free xt/xi/sztmp/cacc

            delta_pool = e2.enter_context(tc.tile_pool(name="delta", bufs=1, side="right"))
            du_pool = e2.enter_context(tc.tile_pool(name="du", bufs=1, side="right"))
            delta_sb = [
                delta_pool.tile([P, L], F32, name=f"dl{d}", tag=f"dl{d}")
                for d in range(NDT)
            ]
            SDT = BF16 if SCAN_BF16 else F32
            du_sb = [
                du_pool.tile([P, L], SDT, name=f"du{d}", tag=f"du{d}")
                for d in range(NDT)
            ]

            # ---- dt_proj + softplus -> delta
            # (no softplus in this build's ACT tables: use ln(1 + exp(z+b)),
            # both funcs live in the natural_log_exp_and_others set)
            sptmp_pool = e1.enter_context(tc.tile_pool(name="sptmp", bufs=2))
            wdt = wpool.tile([P, D_INNER], WDT, name="wdt", tag="wdt", bufs=1)
            nc.sync.dma_start(out=wdt[:DT_RANK, :], in_=dtWT[:])
            for d in range(NDT):
                for th in range(TH):
                    ps = psum.tile([P, 512], F32, name="mm", tag="mm", bufs=3)
                    nc.tensor.matmul(
                        ps[:],
                        wdt[:DT_RANK, d * P : (d + 1) * P],
                        xdbl_sb[:DT_RANK, th * 512 : (th + 1) * 512],
                    )
                    u = sptmp_pool.tile([P, 512], F32, name="spu", tag="spu")
                    nc.scalar.activation(
                        u[:], ps[:], AF.Exp, bias=dtb_sb[:, d : d + 1]
                    )
                    nc.scalar.activation(
                        delta_sb[d][:, th * 512 : (th + 1) * 512],
                        u[:],
                        AF.Ln,
                        bias=1.0,
                    )

            # ---- du = delta * xc (xc still resident)
            for d in range(NDT):
                nc.vector.tensor_mul(du_sb[d][:], delta_sb[d][:], xc_sb[d][:])

            e1.close()  # free w/xcs/xdbl

            # -------- selective scan, s-outer / d-inner
            ypool = e4.enter_context(tc.tile_pool(name="y", bufs=1))
            bcpool = e3.enter_context(tc.tile_pool(name="bc", bufs=2))
            stp = e3.enter_context(tc.tile_pool(name="scantmp", bufs=2))
            hcp = e3.enter_context(tc.tile_pool(name="hCp", bufs=1))

            YDT = BF16 if Y_BF16 else F32
            y_sb = [
                ypool.tile([P, L], YDT, name=f"y{d}", tag=f"y{d}")
                for d in range(NDT)
            ]
            for s in range(D_STATE):
                B_bc = bcpool.tile([P, L], SDT, name="Bbc", tag="Bbc")
                nc.gpsimd.dma_start(out=B_bc[:], in_=_bcast_ap(bc_park[s]))
                C_bc = bcpool.tile([P, L], SDT, name="Cbc", tag="Cbc")
                nc.gpsimd.dma_start(
                    out=C_bc[:], in_=_bcast_ap(bc_park[D_STATE + s])
                )
                for d in range(NDT):
                    dA = stp.tile([P, L], SDT, name="dA", tag="dA")
                    col = d * D_STATE + s
                    nc.scalar.activation(
                        dA[:], delta_sb[d][:], AF.Exp,
                        scale=A_sb[:, col : col + 1],
                    )
                    b = stp.tile([P, L], SDT, name="b", tag="b")
                    nc.vector.tensor_mul(b[:], du_sb[d][:], B_bc[:])
                    h = stp.tile([P, L], SDT, name="h", tag="h")
                    nc.vector.tensor_tensor_scan(
                        h[:], dA[:], b[:], 0.0, op0=OP.mult, op1=OP.add
                    )
                    if s == 0:
                        nc.vector.tensor_mul(y_sb[d][:], h[:], C_bc[:])
                    else:
                        hC = hcp.tile([P, L], SDT, name="hC", tag="hC")
                        nc.vector.tensor_mul(hC[:], h[:], C_bc[:])
                        nc.vector.tensor_add(y_sb[d][:], y_sb[d][:], hC[:])

            e3.close()  # free bc/scantmp/hC
            e2.close()  # free delta/du

            # -------- skip + gate: y3 = (xc*D + y) * silu(z), then projections
            stream = e5.enter_context(tc.tile_pool(name="stream", bufs=2))
            y3pool = e5.enter_context(tc.tile_pool(name="y3", bufs=1))
            mopool = e5.enter_context(tc.tile_pool(name="mo", bufs=1))
            w2pool = e5.enter_context(tc.tile_pool(name="w2", bufs=6))
            otmp = e5.enter_context(tc.tile_pool(name="otmp", bufs=3))

            y3_sb = [
                y3pool.tile([P, L], WDT, name=f"y3{d}", tag=f"y3{d}")
                for d in range(NDT)
            ]
            for d in range(NDT):
                y2 = stream.tile([P, L], F32, name="y2", tag="y2")
                nc.vector.scalar_tensor_tensor(
                    y2[:], xc_sb[d][:], D_sb[:, d : d + 1], y_sb[d][:],
                    op0=OP.mult, op1=OP.add,
                )
                nc.vector.tensor_mul(y3_sb[d][:], y2[:], sz_sb[d][:])

            # ---- out_proj: mo[m,t] = sum_d outWT[d,m]^T y3[d,t]
            mo_sb = [
                mopool.tile([P, L], WDT, name=f"mo{m}", tag=f"mo{m}")
                for m in range(NDM)
            ]
            for m in range(NDM):
                wo = w2pool.tile([P, NDT * P], WDT, name="wo", tag="wo", bufs=2)
                nc.sync.dma_start(
                    out=wo[:], in_=_col_block_ap(outWT, D_MODEL, m * P, P, NDT)
                )
                for th in range(TH):
                    ps = psum.tile([P, 512], F32, name="mm", tag="mm", bufs=3)
                    for d in range(NDT):
                        nc.tensor.matmul(
                            ps[:],
                            wo[:, d * P : (d + 1) * P],
                            y3_sb[d][:, th * 512 : (th + 1) * 512],
                            start=(d == 0),
                            stop=(d == NDT - 1),
                        )
                    nc.scalar.copy(mo_sb[m][:, th * 512 : (th + 1) * 512], ps[:])

            # ---- final half-projection
            for m2 in range(NDM):
                wp = w2pool.tile([P, NDM * P], WDT, name="wp", tag="wp", bufs=2)
                nc.sync.dma_start(
                    out=wp[:], in_=_col_block_ap(projHT, D_MODEL, m2 * P, P, NDM)
                )
                for th in range(TH):
                    ps = psum.tile([P, 512], F32, name="mm", tag="mm", bufs=3)
                    for m in range(NDM):
                        nc.tensor.matmul(
                            ps[:],
                            wp[:, m * P : (m + 1) * P],
                            mo_sb[m][:, th * 512 : (th + 1) * 512],
                            start=(m == 0),
                            stop=(m == NDM - 1),
                        )
                    o = otmp.tile([P, 512], F32, name="o", tag="o")
                    nc.scalar.copy(o[:], ps[:])
                    nc.sync.dma_start(
                        out=part[m2 * P : (m2 + 1) * P, th * 512 : (th + 1) * 512],
                        in_=o[:],
                    )

            e5.close()
            e4.close()

    if split_waits:
        _split_multi_waits(nc)
    return nc


_NC_CACHE = None


def _get_program():
    global _NC_CACHE
    if _NC_CACHE is None:
        _NC_CACHE = _build_program()
    return _NC_CACHE


import ml_dtypes

_BF16_NP = ml_dtypes.bfloat16


# ---------------------------------------------------------------- host glue
def _core_inputs(x_b, pfx, inputs):
    """Build the per-core in_map for one (direction, batch)."""
    c = np.ascontiguousarray
    inW = inputs[pfx + "_inW"]
    convw = inputs[pfx + "_convw"]
    convb = inputs[pfx + "_convb"]
    xprojW = inputs[pfx + "_xprojW"]
    dtW = inputs[pfx + "_dtW"]
    dtb = inputs[pfx + "_dtb"]
    Alog = inputs[pfx + "_Alog"]
    Dv = inputs[pfx + "_D"]
    outW = inputs[pfx + "_outW"]

    A = -np.exp(np.asarray(Alog, np.float32))  # (D_INNER, D_STATE)

    def ptile(v, inner):  # (D_INNER, inner) -> (P, NDT*inner)
        v = np.asarray(v, np.float32).reshape(NDT, P, inner)
        return c(v.transpose(1, 0, 2).reshape(P, NDT * inner))

    return {
        "xT": c(np.asarray(x_b, np.float32).T),
        "inWT": c(np.asarray(inW, np.float32).T),
        "convw": ptile(convw, D_CONV),
        "convb": ptile(np.asarray(convb).reshape(-1, 1), 1),
        "xprojWT": c(np.asarray(xprojW, np.float32).T).astype(_BF16_NP),
        "dtWT": c(np.asarray(dtW, np.float32).T),
        "dtb": ptile(np.asarray(dtb).reshape(-1, 1), 1),
        "Aarr": ptile(A, D_STATE),
        "Dp": ptile(np.asarray(Dv).reshape(-1, 1), 1),
        "outWT": c(np.asarray(outW, np.float32).T),
    }


def _build_in_maps(inputs):
    x = np.asarray(inputs["x"], np.float32)
    projW = np.asarray(inputs["proj_W"], np.float32)
    in_maps = []
    for core in range(N_CORES):
        direction, b = core // B, core % B
        if direction == 0:
            m = _core_inputs(x[b], "fwd", inputs)
            m["projHT"] = np.ascontiguousarray(projW[:, :D_MODEL].T.astype(np.float32))
        else:
            m = _core_inputs(x[b, ::-1], "bwd", inputs)
            m["projHT"] = np.ascontiguousarray(projW[:, D_MODEL:].T.astype(np.float32))
        in_maps.append(m)
    return in_maps


def kernel(**inputs):
    projb = np.asarray(inputs["proj_b"], np.float32)
    in_maps = _build_in_maps(inputs)
    nc = _get_program()
    res = run_bass_kernel_spmd(nc, in_maps, list(range(N_CORES)))

    out = np.empty((B, L, D_MODEL), np.float32)
    for b in range(B):
        pf = res.results[b]["part"]          # (D_MODEL, L)
        pb = res.results[B + b]["part"]      # (D_MODEL, L) in flipped time
        out[b] = (pf + pb[:, ::-1]).T + projb[None, :]
    return out



# revision 26
# speedup vs baseline: 1.4846x; 1.0258x over previous
"""Bidirectional Mamba block on 8 Trainium2 NeuronCores.

Sharding: data-parallel over (direction, batch): core c handles
direction c//4 (0=fwd, 1=bwd) and batch c%4.  The final projection is
linear over the concat([out_fwd, out_bwd]) axis, so each core applies its
direction's half of proj_W and the host sums the two partial outputs
(plus proj_b).  Zero cross-core communication.

Per-core layout is "d-major": tiles are [128 partitions = channel slice,
free = sequence].  The selective-scan recurrence h_t = dA_t*h_{t-1} + dBu_t
runs on the Vector engine's tensor_tensor_scan (prefix scan along the free
dim), once per (state s, channel tile): dA_s = exp(A[:, s] * delta).
"""
import sys

sys.path.insert(0, "/opt/trn_rl_repo")

import numpy as np

import concourse.bass as bass
import concourse.tile as tile
from concourse import mybir
from concourse.bass_utils import run_bass_kernel_spmd
from concourse.vector_clock import ScopedClock

# ---------------------------------------------------------------- shapes
D_MODEL = 768
D_STATE = 16
D_CONV = 4
D_INNER = 1536
DT_RANK = 48
B, L = 4, 1024

P = 128
NDT = D_INNER // P      # 12  channel tiles
NK = D_MODEL // P       # 6   d_model contraction tiles
NE = 2 * D_INNER // P   # 24  in_proj output tiles
NDM = D_MODEL // P      # 6   d_model output tiles
TH = 2                  # two 512-wide t-halves for matmuls
F32 = mybir.dt.float32
AF = mybir.ActivationFunctionType
OP = mybir.AluOpType

N_CORES = 8

BF16 = mybir.dt.bfloat16
F32R = mybir.dt.float32r
SCAN_BF16 = True   # bf16 operands for the scan stage (2x DVE modes)
Y_BF16 = True      # accumulate y in bf16 too (cheaper adds, more error)
MM_DT = "f32r"     # matmul operand dtype: f32 (4 cyc/row), f32r/bf16 (1)
WDT = {"f32": F32, "f32r": F32R, "bf16": BF16}[MM_DT]


def _f32(ap):
    """View a WDT-typed AP as plain fp32 for vector/scalar-engine reads."""
    return ap.bitcast(F32) if MM_DT == "f32r" else ap


MAX_WAITS_PER_INST = 1


class SplitDrainTileContext(tile.TileContext):
    """Walrus in this container rejects >1 sem-wait per instruction; the stock
    kernel-tail drain carries one wait per active processor.  Split them into
    a chain of single-wait SP NOPs."""

    def _drain_and_barrier(self, tick_clock, wait_clock):
        nc = self.nc
        carrier = nc.sync.nop(nofuse=True)
        wait_clock.add_sem_waits(
            carrier.ins, ScopedClock({None: tick_clock.global_clock})
        )
        si = carrier.ins.sync_info
        waits = list(si.on_wait) if si is not None and si.on_wait else []
        if len(waits) > MAX_WAITS_PER_INST:
            carrier.ins.sync_info = mybir.SyncInfo(
                on_wait=waits[:MAX_WAITS_PER_INST], on_update=[]
            )
            rest = waits[MAX_WAITS_PER_INST:]
            for i in range(0, len(rest), MAX_WAITS_PER_INST):
                extra = nc.sync.nop(nofuse=True)
                extra.ins.sync_info = mybir.SyncInfo(
                    on_wait=rest[i : i + MAX_WAITS_PER_INST], on_update=[]
                )
        nc.sync.drain()
        nc.all_engine_barrier()
        assert self.sems is not None
        popped = nc._tile_sem_poison_stack.pop()
        assert popped is self._sem_poison
        nc.clear_and_free_semaphores(list(self.sems.allocated().values()))


def _split_multi_waits(nc):
    """Walrus here accepts at most one sem-wait per instruction.  Tile's
    wait-assignment can attach several (e.g. a matmul waiting on weight DMA +
    rhs producer + PSUM release).  Hoist all but the last wait onto same-
    engine NOPs inserted immediately before the instruction."""
    n = 0
    for fn in nc.m.functions:
        for bb in fn.blocks:
            out = []
            for ins in bb.instructions:
                si = ins.sync_info
                waits = list(si.on_wait) if si is not None and si.on_wait else []
                if len(waits) > 1:
                    for wv in waits[:-1]:
                        nop = mybir.InstNoOp(name=f"wsplit_{n}", ins=[], outs=[])
                        n += 1
                        nop.engine = ins.engine
                        nop.sync_info = mybir.SyncInfo(on_wait=[wv], on_update=[])
                        out.append(nop)
                    ins.sync_info = mybir.SyncInfo(
                        on_wait=[waits[-1]], on_update=list(si.on_update or [])
                    )
                out.append(ins)
            bb.instructions = out
    return n


def _col_block_ap(handle, width, col0, ncols, kcount):
    """AP reading rows [0:128*kcount) x cols [col0:col0+ncols) of a [R, width]
    DRAM tensor as a [128, kcount*ncols] tile (k-blocks side by side)."""
    base = handle[:]
    return bass.AP(
        tensor=base.tensor,
        offset=col0,
        ap=[[width, P], [P * width, kcount], [1, ncols]],
    )


def _bcast_ap(src):
    """AP that reads a [L]-row and broadcasts it across 128 partitions."""
    return bass.AP(
        tensor=src.tensor, offset=src.offset, ap=[[0, P]] + list(src.ap)
    )


# ---------------------------------------------------------------- program
def _build_program(split_waits=True):
    nc = bass.Bass()

    di = lambda name, shape: nc.dram_tensor(name, shape, F32, kind="ExternalInput")
    dw = lambda name, shape: nc.dram_tensor(name, shape, WDT, kind="ExternalInput")
    db = lambda name, shape: nc.dram_tensor(name, shape, BF16, kind="ExternalInput")
    xT = db("xT", [D_MODEL, L])
    inWT = db("inWT", [D_MODEL, 2 * D_INNER])
    convw = di("convw", [P, NDT * D_CONV])   # host pre-tiled [(p), (dt k)]
    convb = di("convb", [P, NDT])
    # bf16: must match the (bf16) xc moving operand — walrus rejects
    # f32r-stationary x bf16-moving matmuls.
    xprojWT = nc.dram_tensor(
        "xprojWT", [D_INNER, DT_RANK + 2 * D_STATE], BF16, kind="ExternalInput"
    )
    dtWT = dw("dtWT", [DT_RANK, D_INNER])
    dtb = di("dtb", [P, NDT])
    Aarr = di("Aarr", [P, NDT * D_STATE])    # host pre-tiled -exp(A_log)
    Dp = di("Dp", [P, NDT])
    outWT = db("outWT", [D_INNER, D_MODEL])
    projHT = db("projHT", [D_MODEL, D_MODEL])

    part = nc.dram_tensor("part", [D_MODEL, L], F32, kind="ExternalOutput")

    bc_park = nc.dram_tensor("bc_park", [2 * D_STATE, L],
                             BF16 if SCAN_BF16 else F32)

    with SplitDrainTileContext(nc) as tc:
        from contextlib import ExitStack

        with ExitStack() as g:
            consts = g.enter_context(tc.tile_pool(name="consts", bufs=1))
            psum = g.enter_context(tc.tile_pool(name="psum", bufs=1, space="PSUM"))

            # -------- constants
            convw_sb = consts.tile([P, NDT * D_CONV], F32, name="convw")
            nc.sync.dma_start(out=convw_sb[:], in_=convw[:])
            convb_sb = consts.tile([P, NDT], F32, name="convb")
            nc.sync.dma_start(out=convb_sb[:], in_=convb[:])
            dtb_sb = consts.tile([P, NDT], F32, name="dtb")
            nc.sync.dma_start(out=dtb_sb[:], in_=dtb[:])
            A_sb = consts.tile([P, NDT * D_STATE], F32, name="A")
            nc.sync.dma_start(out=A_sb[:], in_=Aarr[:])
            D_sb = consts.tile([P, NDT], F32, name="D")
            nc.sync.dma_start(out=D_sb[:], in_=Dp[:])

            e1 = ExitStack()  # [start .. du-end]
            e0 = ExitStack()  # [start .. conv-end]
            e2 = ExitStack()  # [dt_proj .. scan-end]
            e3 = ExitStack()  # [scan .. scan-end]
            e4 = ExitStack()  # [scan .. gate-end]
            e5 = ExitStack()  # [gate .. end]

            # xc and silu(z) stay resident in SBUF as bf16 until the gate
            # stage (e4 scope) instead of round-tripping through DRAM.
            # Allocated before the e0/e1 pools: releases must be LIFO.
            xcs_pool = e4.enter_context(tc.tile_pool(name="xcs", bufs=1))
            sz_pool = e4.enter_context(tc.tile_pool(name="sz", bufs=1))
            wpool = e1.enter_context(tc.tile_pool(name="w", bufs=6))
            xdbl_pool = e1.enter_context(tc.tile_pool(name="xdbl", bufs=1))
            xt_pool = e0.enter_context(tc.tile_pool(name="xt", bufs=1))
            xi_pool = e0.enter_context(tc.tile_pool(name="xi", bufs=1))
            cacc_pool = e0.enter_context(tc.tile_pool(name="cacc", bufs=2))

            # ---- load xT
            xt_sb = []
            for k in range(NK):
                t = xt_pool.tile([P, L], BF16, name=f"xt{k}", tag=f"xt{k}")
                nc.sync.dma_start(out=t[:], in_=xT[k * P : (k + 1) * P, :])
                xt_sb.append(t)

            # ---- in_proj: xzT[e,t] = sum_k inWT[k,e]^T x[k,t]
            xi_sb = [
                xi_pool.tile([P, L + 3], F32, name=f"xi{d}", tag=f"xi{d}")
                for d in range(NDT)
            ]
            for d in range(NDT):
                nc.vector.memset(xi_sb[d][:, 0:3], 0.0)
            sz_sb = [
                sz_pool.tile([P, L], BF16, name=f"szr{d}", tag=f"szr{d}")
                for d in range(NDT)
            ]

            for e in range(NE):
                we = wpool.tile([P, NK * P], BF16, name="we", tag="we", bufs=3)
                nc.sync.dma_start(
                    out=we[:],
                    in_=_col_block_ap(inWT, 2 * D_INNER, e * P, P, NK),
                )
                for th in range(TH):
                    ps = psum.tile([P, 512], F32, name="mm", tag="mm", bufs=3)
                    for k in range(NK):
                        nc.tensor.matmul(
                            ps[:],
                            we[:, k * P : (k + 1) * P],
                            xt_sb[k][:, th * 512 : (th + 1) * 512],
                            start=(k == 0),
                            stop=(k == NK - 1),
                        )
                    if e < NDT:
                        nc.scalar.copy(
                            xi_sb[e][:, 3 + th * 512 : 3 + (th + 1) * 512], ps[:]
                        )
                    else:
                        d = e - NDT
                        nc.scalar.activation(
                            sz_sb[d][:, th * 512 : (th + 1) * 512], ps[:],
                            AF.Silu,
                        )

            # ---- conv + silu -> xc (streamed to DRAM), x_proj accumulates
            # into two PSUM banks held across the d loop
            xdbl_sb = xdbl_pool.tile([P, L], WDT, name="xdbl")
            NR = DT_RANK + 2 * D_STATE  # 80
            psx = [
                psum.tile([P, 512], F32, name=f"mmx{th}", tag=f"mmx{th}")
                for th in range(TH)
            ]
            xc_sb = []
            for d in range(NDT):
                acc = cacc_pool.tile([P, L], F32, name="cacc", tag="cacc")
                nc.vector.tensor_scalar(
                    acc[:],
                    xi_sb[d][:, 0:L],
                    convw_sb[:, 4 * d : 4 * d + 1],
                    convb_sb[:, d : d + 1],
                    op0=OP.mult,
                    op1=OP.add,
                )
                for k in range(1, D_CONV):
                    nc.vector.scalar_tensor_tensor(
                        acc[:],
                        xi_sb[d][:, k : k + L],
                        convw_sb[:, 4 * d + k : 4 * d + k + 1],
                        acc[:],
                        op0=OP.mult,
                        op1=OP.add,
                    )
                xc_t = xcs_pool.tile([P, L], BF16, name=f"xct{d}", tag=f"xct{d}")
                xc_sb.append(xc_t)
                nc.scalar.activation(xc_t[:], acc[:], AF.Silu)

                wx = wpool.tile([P, NR], BF16, name="wx", tag="wx")
                nc.sync.dma_start(out=wx[:], in_=xprojWT[d * P : (d + 1) * P, :])
                for th in range(TH):
                    nc.tensor.matmul(
                        psx[th][:NR, :],
                        wx[:],
                        xc_t[:, th * 512 : (th + 1) * 512],
                        start=(d == 0),
                        stop=(d == NDT - 1),
                    )
            for th in range(TH):
                nc.scalar.copy(
                    xdbl_sb[:NR, th * 512 : (th + 1) * 512], psx[th][:NR, :]
                )

            # park raw B/C rows for broadcast reload
            if SCAN_BF16:
                # ACT partition-window rule: 32-aligned base, <=32 rows per
                # access here.  Two copies cover B (rows 48..63) and C
                # (rows 64..79).
                bcrows = xdbl_pool.tile([64, L], BF16, name="bcrows", bufs=1)
                nc.scalar.copy(bcrows[0:32, :], _f32(xdbl_sb[32:64, :]))
                nc.scalar.copy(bcrows[32:48, :], _f32(xdbl_sb[64:80, :]))
                nc.sync.dma_start(out=bc_park[:], in_=bcrows[16:48, :])
            else:
                nc.sync.dma_start(
                    out=bc_park[:],
                    in_=xdbl_sb[DT_RANK : DT_RANK + 2 * D_STATE, :],
                )

            e0.close()  # free xt/xi/sztmp/cacc

            delta_pool = e2.enter_context(tc.tile_pool(name="delta", bufs=1, side="right"))
            du_pool = e2.enter_context(tc.tile_pool(name="du", bufs=1, side="right"))
            delta_sb = [
                delta_pool.tile([P, L], BF16, name=f"dl{d}", tag=f"dl{d}")
                for d in range(NDT)
            ]
            SDT = BF16 if SCAN_BF16 else F32
            du_sb = [
                du_pool.tile([P, L], SDT, name=f"du{d}", tag=f"du{d}")
                for d in range(NDT)
            ]

            # ---- dt_proj + softplus -> delta
            # (no softplus in this build's ACT tables: use ln(1 + exp(z+b)),
            # both funcs live in the natural_log_exp_and_others set)
            sptmp_pool = e1.enter_context(tc.tile_pool(name="sptmp", bufs=2))
            wdt = wpool.tile([P, D_INNER], WDT, name="wdt", tag="wdt", bufs=1)
            nc.sync.dma_start(out=wdt[:DT_RANK, :], in_=dtWT[:])
            for d in range(NDT):
                for th in range(TH):
                    ps = psum.tile([P, 512], F32, name="mm", tag="mm", bufs=3)
                    nc.tensor.matmul(
                        ps[:],
                        wdt[:DT_RANK, d * P : (d + 1) * P],
                        xdbl_sb[:DT_RANK, th * 512 : (th + 1) * 512],
                    )
                    u = sptmp_pool.tile([P, 512], F32, name="spu", tag="spu")
                    nc.scalar.activation(
                        u[:], ps[:], AF.Exp, bias=dtb_sb[:, d : d + 1]
                    )
                    nc.scalar.activation(
                        delta_sb[d][:, th * 512 : (th + 1) * 512],
                        u[:],
                        AF.Ln,
                        bias=1.0,
                    )

            # ---- du = delta * xc (xc still resident)
            for d in range(NDT):
                nc.vector.tensor_mul(du_sb[d][:], delta_sb[d][:], xc_sb[d][:])

            e1.close()  # free w/xcs/xdbl

            # -------- selective scan, s-outer / d-inner
            ypool = e4.enter_context(tc.tile_pool(name="y", bufs=1))
            bcpool = e3.enter_context(tc.tile_pool(name="bc", bufs=2))
            stp = e3.enter_context(tc.tile_pool(name="scantmp", bufs=2))
            hcp = e3.enter_context(tc.tile_pool(name="hCp", bufs=1))

            YDT = BF16 if Y_BF16 else F32
            y_sb = [
                ypool.tile([P, L], YDT, name=f"y{d}", tag=f"y{d}")
                for d in range(NDT)
            ]
            for s in range(D_STATE):
                B_bc = bcpool.tile([P, L], SDT, name="Bbc", tag="Bbc")
                nc.gpsimd.dma_start(out=B_bc[:], in_=_bcast_ap(bc_park[s]))
                C_bc = bcpool.tile([P, L], SDT, name="Cbc", tag="Cbc")
                nc.gpsimd.dma_start(
                    out=C_bc[:], in_=_bcast_ap(bc_park[D_STATE + s])
                )
                for d in range(NDT):
                    dA = stp.tile([P, L], SDT, name="dA", tag="dA")
                    col = d * D_STATE + s
                    nc.scalar.activation(
                        dA[:], delta_sb[d][:], AF.Exp,
                        scale=A_sb[:, col : col + 1],
                    )
                    b = stp.tile([P, L], SDT, name="b", tag="b")
                    nc.vector.tensor_mul(b[:], du_sb[d][:], B_bc[:])
                    h = stp.tile([P, L], SDT, name="h", tag="h")
                    nc.vector.tensor_tensor_scan(
                        h[:], dA[:], b[:], 0.0, op0=OP.mult, op1=OP.add
                    )
                    if s == 0:
                        nc.vector.tensor_mul(y_sb[d][:], h[:], C_bc[:])
                    else:
                        hC = hcp.tile([P, L], SDT, name="hC", tag="hC")
                        nc.vector.tensor_mul(hC[:], h[:], C_bc[:])
                        nc.vector.tensor_add(y_sb[d][:], y_sb[d][:], hC[:])

            e3.close()  # free bc/scantmp/hC
            e2.close()  # free delta/du

            # -------- skip + gate: y3 = (xc*D + y) * silu(z), then projections
            stream = e5.enter_context(tc.tile_pool(name="stream", bufs=2))
            y3pool = e5.enter_context(tc.tile_pool(name="y3", bufs=1))
            mopool = e5.enter_context(tc.tile_pool(name="mo", bufs=1))
            w2pool = e5.enter_context(tc.tile_pool(name="w2", bufs=6))
            otmp = e5.enter_context(tc.tile_pool(name="otmp", bufs=3))

            y3_sb = [
                y3pool.tile([P, L], BF16, name=f"y3{d}", tag=f"y3{d}")
                for d in range(NDT)
            ]
            for d in range(NDT):
                y2 = stream.tile([P, L], BF16, name="y2", tag="y2")
                nc.vector.scalar_tensor_tensor(
                    y2[:], xc_sb[d][:], D_sb[:, d : d + 1], y_sb[d][:],
                    op0=OP.mult, op1=OP.add,
                )
                nc.vector.tensor_mul(y3_sb[d][:], y2[:], sz_sb[d][:])

            # ---- out_proj: mo[m,t] = sum_d outWT[d,m]^T y3[d,t]
            mo_sb = [
                mopool.tile([P, L], BF16, name=f"mo{m}", tag=f"mo{m}")
                for m in range(NDM)
            ]
            for m in range(NDM):
                wo = w2pool.tile([P, NDT * P], BF16, name="wo", tag="wo", bufs=2)
                nc.sync.dma_start(
                    out=wo[:], in_=_col_block_ap(outWT, D_MODEL, m * P, P, NDT)
                )
                for th in range(TH):
                    ps = psum.tile([P, 512], F32, name="mm", tag="mm", bufs=3)
                    for d in range(NDT):
                        nc.tensor.matmul(
                            ps[:],
                            wo[:, d * P : (d + 1) * P],
                            y3_sb[d][:, th * 512 : (th + 1) * 512],
                            start=(d == 0),
                            stop=(d == NDT - 1),
                        )
                    nc.scalar.copy(mo_sb[m][:, th * 512 : (th + 1) * 512], ps[:])

            # ---- final half-projection
            for m2 in range(NDM):
                wp = w2pool.tile([P, NDM * P], BF16, name="wp", tag="wp", bufs=2)
                nc.sync.dma_start(
                    out=wp[:], in_=_col_block_ap(projHT, D_MODEL, m2 * P, P, NDM)
                )
                for th in range(TH):
                    ps = psum.tile([P, 512], F32, name="mm", tag="mm", bufs=3)
                    for m in range(NDM):
                        nc.tensor.matmul(
                            ps[:],
                            wp[:, m * P : (m + 1) * P],
                            mo_sb[m][:, th * 512 : (th + 1) * 512],
                            start=(m == 0),
                            stop=(m == NDM - 1),
                        )
                    o = otmp.tile([P, 512], F32, name="o", tag="o")
                    nc.scalar.copy(o[:], ps[:])
                    nc.sync.dma_start(
                        out=part[m2 * P : (m2 + 1) * P, th * 512 : (th + 1) * 512],
                        in_=o[:],
                    )

            e5.close()
            e4.close()

    if split_waits:
        _split_multi_waits(nc)
    return nc


_NC_CACHE = None


def _get_program():
    global _NC_CACHE
    if _NC_CACHE is None:
        _NC_CACHE = _build_program()
    return _NC_CACHE


import ml_dtypes

_BF16_NP = ml_dtypes.bfloat16


# ---------------------------------------------------------------- host glue
def _core_inputs(x_b, pfx, inputs):
    """Build the per-core in_map for one (direction, batch)."""
    c = np.ascontiguousarray
    inW = inputs[pfx + "_inW"]
    convw = inputs[pfx + "_convw"]
    convb = inputs[pfx + "_convb"]
    xprojW = inputs[pfx + "_xprojW"]
    dtW = inputs[pfx + "_dtW"]
    dtb = inputs[pfx + "_dtb"]
    Alog = inputs[pfx + "_Alog"]
    Dv = inputs[pfx + "_D"]
    outW = inputs[pfx + "_outW"]

    A = -np.exp(np.asarray(Alog, np.float32))  # (D_INNER, D_STATE)

    def ptile(v, inner):  # (D_INNER, inner) -> (P, NDT*inner)
        v = np.asarray(v, np.float32).reshape(NDT, P, inner)
        return c(v.transpose(1, 0, 2).reshape(P, NDT * inner))

    return {
        "xT": c(np.asarray(x_b, np.float32).T).astype(_BF16_NP),
        "inWT": c(np.asarray(inW, np.float32).T).astype(_BF16_NP),
        "convw": ptile(convw, D_CONV),
        "convb": ptile(np.asarray(convb).reshape(-1, 1), 1),
        "xprojWT": c(np.asarray(xprojW, np.float32).T).astype(_BF16_NP),
        "dtWT": c(np.asarray(dtW, np.float32).T),
        "dtb": ptile(np.asarray(dtb).reshape(-1, 1), 1),
        "Aarr": ptile(A, D_STATE),
        "Dp": ptile(np.asarray(Dv).reshape(-1, 1), 1),
        "outWT": c(np.asarray(outW, np.float32).T).astype(_BF16_NP),
    }


def _build_in_maps(inputs):
    x = np.asarray(inputs["x"], np.float32)
    projW = np.asarray(inputs["proj_W"], np.float32)
    in_maps = []
    for core in range(N_CORES):
        direction, b = core // B, core % B
        if direction == 0:
            m = _core_inputs(x[b], "fwd", inputs)
            m["projHT"] = np.ascontiguousarray(projW[:, :D_MODEL].T).astype(_BF16_NP)
        else:
            m = _core_inputs(x[b, ::-1], "bwd", inputs)
            m["projHT"] = np.ascontiguousarray(projW[:, D_MODEL:].T).astype(_BF16_NP)
        in_maps.append(m)
    return in_maps


def kernel(**inputs):
    projb = np.asarray(inputs["proj_b"], np.float32)
    in_maps = _build_in_maps(inputs)
    nc = _get_program()
    res = run_bass_kernel_spmd(nc, in_maps, list(range(N_CORES)))

    out = np.empty((B, L, D_MODEL), np.float32)
    for b in range(B):
        pf = res.results[b]["part"]          # (D_MODEL, L)
        pb = res.results[B + b]["part"]      # (D_MODEL, L) in flipped time
        out[b] = (pf + pb[:, ::-1]).T + projb[None, :]
    return out

